# revision 22
# baseline (speedup 1.0000x reference)
"""GAT layer (gnn_message_passing) Bass kernel for 8 Trainium2 NeuronCores.

Row-sharded: core c computes output rows [c*R, (c+1)*R) of
    out = softmax(mask(leakyrelu(s_src[i]+s_dst[j]), adj)) @ (h @ W.T)

v3 design notes (HW-measured op costs drove every choice):
  - All PE traffic is bf16 (fp32 matmul = 4 cyc/col, bf16 = 1). ldw-opt must
    stay disabled: walrus rejects Tile-pre-split bf16 LDWEIGHTS under it.
  - Per [128,1024] bf16 tile on HW: DVE tensor_scalar = 427ns (4x mode, even
    with a per-partition AP scalar), tensor_tensor = 692ns (2x),
    scalar_tensor_tensor = 1225ns (1x only - avoid), ACT op = 1147ns,
    batched ACT exp = 927ns/chunk, Pool TT = 2117ns, Pool TS = 14.7us(!).
  - The adjacency mask is applied by the DMA engine: madj in {0, -64} as
    fp8e4, SWDGE-accumulated (accum_op=add) straight into the leakyrelu
    output tile before the exp. exp(prelu(e)-64) ~ 1e-27 -> exact-enough 0.
    One accum-DMA per 4 chunks (host pre-arranges the mask so a [128, 4096]
    slice matches the batch tile) costs ~1.2us of Pool sequencer time.
  - leakyrelu(e) = max(e, 0.2e) with e = s_src[i]+s_dst[j] is built from
    resident tensors only: e1 = TS(ssrc + sdst[j]), e2 = TS-dual
    ((ssrc + sdst[j]) * 0.2), max = TT. The TT-max alternates DVE/Pool and
    1/16 of chunks run the whole thing as one ACT Prelu (bias+alpha fused)
    to balance the three engines.
  - Unnormalized softmax (|e| <= ~4): out_i = (p @ Wh)_i / sum_j p[i,j];
    row sums via a second accumulating matmul with a ones stationary.

Layout: transposed on device, [j (source node) on partitions, i (dest node)
on free]. p.T tiles feed the TensorEngine directly as moving operands for
outT += Wh[jc].T @ pT with zero on-chip transposes.
"""

import functools
import sys

sys.path.insert(0, "/opt/trn_rl_repo")

import numpy as np
import ml_dtypes

import bass_rust
import concourse.bass as bass
import concourse.mybir as mybir
import concourse.tile as tile
from concourse.masks import make_identity
from concourse.bass_utils import run_bass_kernel_spmd

F32 = mybir.dt.float32
BF16 = mybir.dt.bfloat16
FP8 = mybir.dt.float8e4
AF = mybir.ActivationFunctionType
ALU = mybir.AluOpType

N_CORES = 8
MASK_NEG = -64.0  # added to leakyrelu(e) where adj==0; exp(x-64) ~ 0


def _patch_tail_drain():
    """This walrus build caps sync waits at 1 per instruction (2 for EVSEM),
    but Tile emits multi-wait instructions in two places: regular insts via
    assign_waits, and the tail drain. Split surplus waits onto same-engine
    wait-only NOPs placed immediately before (regular) / after (tail drain)
    the owning instruction."""
    from concourse.tile import ScopedClock, TileContext

    if getattr(TileContext, "_drain_patched", False):
        return

    _orig_loi = TileContext._lower_ordered_insts

    def _lower_ordered_insts(self, ordered):
        nc = self.nc
        ws_id = 0
        for bbname in list(ordered.keys()):
            insts = ordered[bbname]
            new = []
            for inst in insts:
                si = inst.sync_info
                if si is not None:
                    cap = 2 if isinstance(inst, mybir.InstEventSemaphore) else 1
                    waits = list(si.on_wait)
                    if len(waits) > cap:
                        extra, keep = waits[:-cap], waits[-cap:]
                        for w in extra:
                            nop = mybir.InstNoOp(
                                name=f"{inst.name}-ws{ws_id}", ins=[], outs=[]
                            )
                            ws_id += 1
                            nop.engine = inst.engine
                            nop.sync_info = bass_rust.SyncInfo(
                                on_wait=[w], on_update=[]
                            )
                            nc.register_instruction(nop, overwrite=True)
                            new.append(nop)
                        inst.sync_info = bass_rust.SyncInfo(
                            on_wait=keep, on_update=list(si.on_update)
                        )
                new.append(inst)
            ordered[bbname] = new
        return _orig_loi(self, ordered)

    TileContext._lower_ordered_insts = _lower_ordered_insts

    def _drain_and_barrier(self, tick_clock, wait_clock):
        drain_inst = self.nc.sync.drain()
        wait_clock.add_sem_waits(
            drain_inst.ins, ScopedClock({None: tick_clock.global_clock})
        )
        si = drain_inst.ins.sync_info
        if si is not None and len(si.on_wait) > 1:
            waits = list(si.on_wait)
            drain_inst.ins.sync_info = bass_rust.SyncInfo(
                on_wait=[waits[0]], on_update=list(si.on_update)
            )
            for w in waits[1:]:
                nop = self.nc.sync.nop(nofuse=True)
                nop.ins.sync_info = bass_rust.SyncInfo(on_wait=[w], on_update=[])
        self.nc.all_engine_barrier()
        assert self.sems is not None
        popped = self.nc._tile_sem_poison_stack.pop()
        assert popped is self._sem_poison
        self.nc.clear_and_free_semaphores(list(self.sems.allocated().values()))
        self.nc.all_engine_barrier()

    TileContext._drain_and_barrier = _drain_and_barrier
    TileContext._drain_patched = True


def build_gat_nc(N=8192, R=1024, FIN=256, FOUT=128):
    """Build the per-core Bass program (transposed layout). All cores run the
    same program on different data slices."""
    import os

    # bisection knobs (default = fastest path)
    swdge_split = int(os.environ.get("GAT_SWDGE_SPLIT", "2"))  # chunks per accum DMA (4=whole group fails >4KB/partition)
    no_pool_tt = bool(int(os.environ.get("GAT_NO_POOL_TT", "0")))
    no_dma_mask = bool(int(os.environ.get("GAT_NO_DMA_MASK", "0")))
    _patch_tail_drain()

    P = 128
    FK = FIN // P          # fin chunks (contraction for Wh)
    NCH = N // P           # 128-row j-chunks over all N source nodes
    RB = R // P            # 128-wide i-subblocks per core
    SEG = 512 if R % 512 == 0 else R
    NSEG = R // SEG
    EB = 2 if NCH % 2 == 0 else 1   # chunks per batched Exp / mask-DMA group
    WB = 2 if NCH % 2 == 0 else 1   # Wh chunks per PSUM tile

    nc = bass.Bass()
    hT_t = nc.dram_tensor("hT", [FIN, N], BF16, kind="ExternalInput")
    hTo_t = nc.dram_tensor("hT_own", [FIN, R], BF16, kind="ExternalInput")
    # mask, fp8 {0,-64}, pre-arranged so group G lives at rows [G*128,(G+1)*128)
    # with the EB chunks of the group concatenated along the free dim.
    madj_t = nc.dram_tensor("madj8", [(NCH // EB) * P, EB * R], FP8, kind="ExternalInput")
    w_t = nc.dram_tensor("W", [FOUT, FIN], F32, kind="ExternalInput")
    wT_t = nc.dram_tensor("WT", [FIN, FOUT], BF16, kind="ExternalInput")
    a_t = nc.dram_tensor("a", [2 * FOUT, 1], F32, kind="ExternalInput")
    out_t = nc.dram_tensor("out_blk", [R, FOUT], F32, kind="ExternalOutput")

    with tile.TileContext(nc) as tc:
        with tc.tile_pool(name="persist", bufs=1) as persist:
            ident = persist.tile([P, P], F32)
            make_identity(nc, ident)
            ones_col = persist.tile([P, 1], BF16)
            nc.vector.memset(ones_col, 1.0)
            ones_row = persist.tile([1, P], BF16)
            nc.vector.memset(ones_row, 1.0)
            hT_sb = persist.tile([P, FK, N], BF16)       # h.T, fin on partitions
            hTo_sb = persist.tile([P, FK, R], BF16)      # own rows of h.T
            whs_sb = persist.tile([P, NCH, FOUT], BF16)  # Wh, j on partitions
            sdst_col = persist.tile([P, NCH, 1], F32)    # s_dst, partition-major
            ssrc_bcast = persist.tile([P, R], BF16)      # s_src bcast to all partitions
            ssrc02_bcast = persist.tile([P, R], BF16)    # 0.2 * s_src bcast
            sdst02_col = persist.tile([P, NCH, 1], F32)  # 0.2 * s_dst
            rhs_aug = persist.tile([P, FK, FOUT + 1], BF16)  # [W.T | w_dst] per fin chunk
            wsrc_sb = persist.tile([P, FK], BF16)        # w_src per fin chunk

            # startup DMAs: spread dispatch across engine sequencers (each
            # HWDGE dispatch costs ~600ns of sequencer time; serializing 20+
            # of them on SP alone wasted 14us of startup).
            for k in range(FK):
                nc.scalar.dma_start(out=hTo_sb[:, k, :], in_=hTo_t[k * P : (k + 1) * P, :])
                nc.scalar.dma_start(
                    out=rhs_aug[:, k, 0:FOUT], in_=wT_t[k * P : (k + 1) * P, :]
                )
            HPC = N // 2 if N % 2 == 0 else N
            for c0 in range(0, N, HPC):
                for k in range(FK):
                    nc.sync.dma_start(
                        out=hT_sb[:, k, c0 : c0 + HPC],
                        in_=hT_t[k * P : (k + 1) * P, c0 : c0 + HPC],
                    )

            # ---------------- prologue: w_src/w_dst, s_src ----------------
            with (
                tc.tile_pool(name="pro", bufs=1) as pro,
                tc.tile_pool(name="pro_ps", bufs=1, space="PSUM") as pro_ps,
            ):
                w_sb = pro.tile([P, FIN], F32)
                nc.scalar.dma_start(out=w_sb, in_=w_t[:, :])
                acol = pro.tile([P, 2], F32)
                nc.scalar.dma_start(out=acol[:, 0:1], in_=a_t[0:FOUT, :])        # a_src
                nc.scalar.dma_start(out=acol[:, 1:2], in_=a_t[FOUT : 2 * FOUT, :])  # a_dst

                for k in range(FK):
                    wchunk = w_sb[:, k * P : (k + 1) * P]
                    pw = pro_ps.tile([P, 2], F32, tag="wv")
                    nc.tensor.matmul(pw[:, 0:1], wchunk, acol[:, 1:2], start=True, stop=True)
                    nc.tensor.matmul(pw[:, 1:2], wchunk, acol[:, 0:1], start=True, stop=True)
                    nc.vector.tensor_copy(out=rhs_aug[:, k, FOUT : FOUT + 1], in_=pw[:, 0:1])
                    nc.vector.tensor_copy(out=wsrc_sb[:, k : k + 1], in_=pw[:, 1:2])

                # s_src for own rows (bf16 operands, fp32 PSUM accumulate)
                sp = pro_ps.tile([P, RB], F32, tag="sp")
                for b in range(RB):
                    for k in range(FK):
                        nc.tensor.matmul(
                            sp[:, b : b + 1],
                            hTo_sb[:, k, b * P : (b + 1) * P],
                            wsrc_sb[:, k : k + 1],
                            start=(k == 0),
                            stop=(k == FK - 1),
                        )
                ssrc_col = pro.tile([P, RB], F32)
                nc.vector.tensor_copy(out=ssrc_col, in_=sp)

                # s_src broadcast across partitions: per-partition columns ->
                # one row (PE transposes), then outer-product with ones (K=1
                # matmul) to replicate down the partition dim.
                srow_ps = pro_ps.tile([1, R], F32, tag="srow")
                for b in range(RB):
                    nc.tensor.transpose(
                        srow_ps[:, b * P : (b + 1) * P], ssrc_col[:, b : b + 1], ident
                    )
                srow_sb = pro.tile([1, R], BF16)
                nc.vector.tensor_copy(out=srow_sb, in_=srow_ps)
                sbc_ps = pro_ps.tile([P, R], F32, tag="sbc")
                BSEG = 512 if R % 512 == 0 else R
                for s in range(R // BSEG):
                    nc.tensor.matmul(
                        sbc_ps[:, s * BSEG : (s + 1) * BSEG],
                        ones_row,
                        srow_sb[:, s * BSEG : (s + 1) * BSEG],
                        start=True,
                        stop=True,
                    )
                nc.vector.tensor_copy(out=ssrc_bcast, in_=sbc_ps)
                nc.vector.tensor_scalar(
                    out=ssrc02_bcast, in0=ssrc_bcast, scalar1=0.2, scalar2=None,
                    op0=ALU.mult,
                )

            # ------------- main: Wh chunks interleaved with attention -------------
            with (
                tc.tile_pool(name="whp", bufs=2, space="PSUM") as whp,
                tc.tile_pool(name="e1p", bufs=6) as e1p,
                tc.tile_pool(name="e2p", bufs=6) as e2p,
                tc.tile_pool(name="xbp", bufs=4) as xbp,
                tc.tile_pool(name="pqp", bufs=6) as pqp,
                tc.tile_pool(name="sm", bufs=2) as sm,
                tc.tile_pool(name="osb", bufs=2) as osb,
                tc.tile_pool(name="out_ps", bufs=1, space="PSUM") as out_ps,
                tc.tile_pool(name="tr_ps", bufs=1, space="PSUM") as tr_ps,
            ):
                psum_outT = [
                    out_ps.tile([P, SEG], F32, tag=f"poT{s}", name=f"poT{s}")
                    for s in range(NSEG)
                ]
                psum_sums = [
                    out_ps.tile([1, SEG], F32, tag=f"psm{s}", name=f"psm{s}")
                    for s in range(NSEG)
                ]
                xb = None
                pending = None
                flush_ref = []
                # ---- Wh phase: all chunks up front (fills the startup DMA
                # window; keeps PSUM-copy traffic out of the attention loop
                # so the DVE/ACT FIFOs never stall on cross-phase deps) ----
                for c2 in range(NCH // WB):
                    wh_ps = whp.tile([P, WB, FOUT + 1], F32, tag="wh", name="wh_ps")
                    for i in range(WB):
                        c = c2 * WB + i
                        for k in range(FK):
                            nc.tensor.matmul(
                                wh_ps[:, i, :],
                                hT_sb[:, k, c * P : (c + 1) * P],
                                rhs_aug[:, k, :],
                                start=(k == 0),
                                stop=(k == FK - 1),
                            )
                    if c2 % 2 == 0:
                        nc.vector.tensor_copy(
                            out=whs_sb[:, c2 * WB : (c2 + 1) * WB, :],
                            in_=wh_ps[:, :, 0:FOUT],
                        )
                    else:
                        nc.scalar.activation(
                            out=whs_sb[:, c2 * WB : (c2 + 1) * WB, :],
                            in_=wh_ps[:, :, 0:FOUT],
                            func=AF.Copy,
                            bias=0.0,
                        )
                    nc.scalar.activation(
                        out=sdst_col[:, c2 * WB : (c2 + 1) * WB, :],
                        in_=wh_ps[:, :, FOUT : FOUT + 1],
                        func=AF.Copy,
                        bias=0.0,
                    )
                    nc.scalar.activation(
                        out=sdst02_col[:, c2 * WB : (c2 + 1) * WB, :],
                        in_=wh_ps[:, :, FOUT : FOUT + 1],
                        func=AF.Copy,
                        bias=0.0,
                        scale=0.2,
                    )

                # ---- attention loop ----
                for c2 in range(NCH // WB):
                    def flush_group(jc_last, xbt):
                        """Mask + exp + matmuls for the EB-chunk group ending
                        at jc_last. Emitted one group late (software pipeline)
                        so the Pool/ACT FIFOs never head-of-line block on the
                        mask DMA's dependencies."""
                        grp = jc_last // EB
                        if no_dma_mask:
                            mtl = e1p.tile([P, EB, R], FP8, tag="mt", name="mtl")
                            nc.sync.dma_start(
                                out=mtl, in_=madj_t[grp * P : (grp + 1) * P, :]
                            )
                            for gg in range(EB):
                                nc.vector.tensor_tensor(
                                    out=xbt[:, gg, :], in0=xbt[:, gg, :],
                                    in1=mtl[:, gg, :], op=ALU.add,
                                )
                        else:
                            sw = swdge_split if swdge_split else EB
                            for gg in range(0, EB, sw):
                                nc.gpsimd.dma_start(
                                    out=xbt[:, gg : gg + sw, :],
                                    in_=madj_t[
                                        grp * P : (grp + 1) * P,
                                        gg * R : (gg + sw) * R,
                                    ],
                                    accum_op=ALU.add,
                                )
                        pq = pqp.tile([P, EB, R], BF16, tag="pq", name="pq")
                        nc.scalar.activation(out=pq, in_=xbt, func=AF.Exp)
                        jc0 = jc_last - (EB - 1)
                        for gg in range(EB):
                            jcc = jc0 + gg
                            for s in range(NSEG):
                                nc.tensor.matmul(
                                    psum_outT[s],
                                    whs_sb[:, jcc, :],
                                    pq[:, gg, s * SEG : (s + 1) * SEG],
                                    start=(jcc == 0),
                                    stop=(jcc == NCH - 1),
                                )
                        for gg in range(EB):
                            jcc = jc0 + gg
                            for s in range(NSEG):
                                nc.tensor.matmul(
                                    psum_sums[s],
                                    ones_col,
                                    pq[:, gg, s * SEG : (s + 1) * SEG],
                                    start=(jcc == 0),
                                    stop=(jcc == NCH - 1),
                                )

                    flush_ref[:] = [flush_group]

                    for i in range(WB):
                        jc = c2 * WB + i
                        g = jc % EB
                        if g == 0:
                            xb = xbp.tile([P, EB, R], BF16, tag="xb", name="xb")
                        if jc % 16 in (3, 7, 11):
                            # balance valve: full leakyrelu on ACT (bias and
                            # alpha fused into one ACTIVATE)
                            nc.scalar.activation(
                                out=xb[:, g, :],
                                in_=ssrc_bcast,
                                func=AF.Prelu,
                                bias=sdst_col[:, jc, :],
                                scale=1.0,
                                alpha=0.2,
                            )
                        else:
                            # e1 = s_src + s_dst[j] (Pool TT-add with a
                            # free-broadcast [P,1] operand for 4/16 chunks,
                            # DVE TS otherwise); e2 = 0.2*e1 (single-op TS);
                            # leakyrelu = max(e1, e2) on DVE.
                            e1 = e1p.tile([P, R], BF16, tag="e1", name="e1")
                            if not no_pool_tt and jc % 16 in (0, 2, 5, 9, 13):
                                nc.gpsimd.tensor_tensor(
                                    out=e1,
                                    in0=ssrc_bcast,
                                    in1=bass.broadcast_tensor_aps(
                                        ssrc_bcast[:, :], sdst_col[:, jc, :]
                                    )[1],
                                    op=ALU.add,
                                )
                            else:
                                nc.vector.tensor_scalar(
                                    out=e1,
                                    in0=ssrc_bcast,
                                    scalar1=sdst_col[:, jc, :],
                                    scalar2=None,
                                    op0=ALU.add,
                                )
                            e2 = e2p.tile([P, R], BF16, tag="e2", name="e2")
                            nc.vector.tensor_scalar(
                                out=e2,
                                in0=ssrc02_bcast,
                                scalar1=sdst02_col[:, jc, :],
                                scalar2=None,
                                op0=ALU.add,
                            )
                            nc.vector.tensor_tensor(
                                out=xb[:, g, :], in0=e1, in1=e2, op=ALU.max
                            )
                        if g != EB - 1:
                            continue
                        if pending is not None:
                            flush_group(*pending)
                        pending = (jc, xb)

                if pending is not None:
                    flush_ref[0](*pending)

                # tail: denominators back to per-partition layout, transpose
                # out.T blocks, scale, store.
                sums_sb = sm.tile([1, R], F32, tag="ssb", name="sums_sb")
                for s in range(NSEG):
                    nc.vector.tensor_copy(
                        out=sums_sb[:, s * SEG : (s + 1) * SEG], in_=psum_sums[s]
                    )
                # [1, R] row -> [P, RB] per-partition columns via tiny PE
                # transposes ([1,128].T @ [[1]] = [128,1]).
                rsums_ps = tr_ps.tile([P, RB], F32, tag="rs", name="rsums_ps")
                for b in range(RB):
                    nc.tensor.transpose(
                        rsums_ps[:, b : b + 1],
                        sums_sb[0:1, b * P : (b + 1) * P],
                        ident[0:1, 0:1],
                    )
                recip_col = sm.tile([P, RB], F32, tag="rcc", name="recip_col")
                nc.vector.reciprocal(recip_col, rsums_ps)
                outT_sb = sm.tile([P, R], F32, tag="oT", name="outT_sb")
                for s in range(NSEG):
                    nc.vector.tensor_copy(
                        out=outT_sb[:, s * SEG : (s + 1) * SEG], in_=psum_outT[s]
                    )
                for b in range(RB):
                    tr = tr_ps.tile([P, P], F32, tag="tr", name="tr")
                    nc.tensor.transpose(
                        tr, outT_sb[:, b * P : (b + 1) * P], ident
                    )
                    out_sb = osb.tile([P, FOUT], F32, tag="ob", name="out_sb")
                    nc.scalar.activation(
                        out=out_sb,
                        in_=tr,
                        func=AF.Copy,
                        bias=0.0,
                        scale=recip_col[:, b : b + 1],
                    )
                    nc.sync.dma_start(out=out_t[b * P : (b + 1) * P, :], in_=out_sb)

    return nc


@functools.lru_cache(maxsize=2)
def _compiled(N, R, FIN, FOUT):
    return build_gat_nc(N=N, R=R, FIN=FIN, FOUT=FOUT)


def run_gat(h, adj, W, a, trace=False, tmpdir=None):
    BF = ml_dtypes.bfloat16
    E4 = ml_dtypes.float8_e4m3
    h = np.asarray(h, dtype=np.float32)
    adj = np.asarray(adj, dtype=np.int32)
    N, FIN = h.shape
    FOUT = np.asarray(W).shape[0]
    R = N // N_CORES
    P = 128
    NCH = N // P
    EB = 2 if NCH % 2 == 0 else 1
    nc = _compiled(N, R, FIN, FOUT)

    hT_bf = np.ascontiguousarray(h.T.astype(BF))
    W32 = np.ascontiguousarray(np.asarray(W, dtype=np.float32))
    WT_bf = np.ascontiguousarray(W32.T.astype(BF))
    a32 = np.ascontiguousarray(np.asarray(a, dtype=np.float32).reshape(2 * FOUT, 1))
    # additive mask: adj==1 -> 0.0, adj==0 -> MASK_NEG, fp8_e4m3
    lut = np.array([MASK_NEG, 0.0], dtype=E4)

    in_maps = []
    for c in range(N_CORES):
        sl = slice(c * R, (c + 1) * R)
        madjT = lut[adj[sl].T]                    # [N, R] fp8 {0,-64}
        # group-major layout: [NCH//EB, EB, P, R] -> [NCH//EB, P, EB, R]
        m8 = (
            madjT.reshape(NCH // EB, EB, P, R)
            .transpose(0, 2, 1, 3)
            .reshape((NCH // EB) * P, EB * R)
        )
        in_maps.append(
            {
                "hT": hT_bf,
                "hT_own": np.ascontiguousarray(h[sl].T.astype(BF)),
                "madj8": np.ascontiguousarray(m8),
                "W": W32,
                "WT": WT_bf,
                "a": a32,
            }
        )
    res = run_bass_kernel_spmd(
        nc, in_maps, core_ids=list(range(N_CORES)), trace=trace, tmpdir=tmpdir
    )
    out = np.concatenate([r["out_blk"] for r in res.results], axis=0)
    return out, res


def kernel(h, adj, W, a):
    out, _ = run_gat(np.asarray(h), np.asarray(adj), np.asarray(W), np.asarray(a))
    return out.astype(np.float32)


# revision 24
# speedup vs baseline: 1.1351x; 1.1351x over previous
"""GAT layer (gnn_message_passing) Bass kernel for 8 Trainium2 NeuronCores.

Row-sharded: core c computes output rows [c*R, (c+1)*R) of
    out = softmax(mask(leakyrelu(s_src[i]+s_dst[j]), adj)) @ (h @ W.T)

v3 design notes (HW-measured op costs drove every choice):
  - All PE traffic is bf16 (fp32 matmul = 4 cyc/col, bf16 = 1). ldw-opt must
    stay disabled: walrus rejects Tile-pre-split bf16 LDWEIGHTS under it.
  - Per [128,1024] bf16 tile on HW: DVE tensor_scalar = 427ns (4x mode, even
    with a per-partition AP scalar), tensor_tensor = 692ns (2x),
    scalar_tensor_tensor = 1225ns (1x only - avoid), ACT op = 1147ns,
    batched ACT exp = 927ns/chunk, Pool TT = 2117ns, Pool TS = 14.7us(!).
  - The adjacency mask is applied by the DMA engine: madj in {0, -64} as
    fp8e4, SWDGE-accumulated (accum_op=add) straight into the leakyrelu
    output tile before the exp. exp(prelu(e)-64) ~ 1e-27 -> exact-enough 0.
    One accum-DMA per 4 chunks (host pre-arranges the mask so a [128, 4096]
    slice matches the batch tile) costs ~1.2us of Pool sequencer time.
  - leakyrelu(e) = max(e, 0.2e) with e = s_src[i]+s_dst[j] is built from
    resident tensors only: e1 = TS(ssrc + sdst[j]), e2 = TS-dual
    ((ssrc + sdst[j]) * 0.2), max = TT. The TT-max alternates DVE/Pool and
    1/16 of chunks run the whole thing as one ACT Prelu (bias+alpha fused)
    to balance the three engines.
  - Unnormalized softmax (|e| <= ~4): out_i = (p @ Wh)_i / sum_j p[i,j];
    row sums via a second accumulating matmul with a ones stationary.

Layout: transposed on device, [j (source node) on partitions, i (dest node)
on free]. p.T tiles feed the TensorEngine directly as moving operands for
outT += Wh[jc].T @ pT with zero on-chip transposes.
"""

import functools
import sys

sys.path.insert(0, "/opt/trn_rl_repo")

import numpy as np
import ml_dtypes

import bass_rust
import concourse.bass as bass
import concourse.mybir as mybir
import concourse.tile as tile
from concourse.masks import make_identity
from concourse.bass_utils import run_bass_kernel_spmd

F32 = mybir.dt.float32
BF16 = mybir.dt.bfloat16
FP8 = mybir.dt.float8e4
AF = mybir.ActivationFunctionType
ALU = mybir.AluOpType

N_CORES = 8
MASK_NEG = -64.0  # added to leakyrelu(e) where adj==0; exp(x-64) ~ 0


def _patch_tail_drain():
    """This walrus build caps sync waits at 1 per instruction (2 for EVSEM),
    but Tile emits multi-wait instructions in two places: regular insts via
    assign_waits, and the tail drain. Split surplus waits onto same-engine
    wait-only NOPs placed immediately before (regular) / after (tail drain)
    the owning instruction."""
    from concourse.tile import ScopedClock, TileContext

    if getattr(TileContext, "_drain_patched", False):
        return

    _orig_loi = TileContext._lower_ordered_insts

    def _lower_ordered_insts(self, ordered):
        nc = self.nc
        ws_id = 0
        for bbname in list(ordered.keys()):
            insts = ordered[bbname]
            new = []
            for inst in insts:
                si = inst.sync_info
                if si is not None:
                    cap = 2 if isinstance(inst, mybir.InstEventSemaphore) else 1
                    waits = list(si.on_wait)
                    if len(waits) > cap:
                        extra, keep = waits[:-cap], waits[-cap:]
                        for w in extra:
                            nop = mybir.InstNoOp(
                                name=f"{inst.name}-ws{ws_id}", ins=[], outs=[]
                            )
                            ws_id += 1
                            nop.engine = inst.engine
                            nop.sync_info = bass_rust.SyncInfo(
                                on_wait=[w], on_update=[]
                            )
                            nc.register_instruction(nop, overwrite=True)
                            new.append(nop)
                        inst.sync_info = bass_rust.SyncInfo(
                            on_wait=keep, on_update=list(si.on_update)
                        )
                new.append(inst)
            ordered[bbname] = new
        return _orig_loi(self, ordered)

    TileContext._lower_ordered_insts = _lower_ordered_insts

    def _drain_and_barrier(self, tick_clock, wait_clock):
        drain_inst = self.nc.sync.drain()
        wait_clock.add_sem_waits(
            drain_inst.ins, ScopedClock({None: tick_clock.global_clock})
        )
        si = drain_inst.ins.sync_info
        if si is not None and len(si.on_wait) > 1:
            waits = list(si.on_wait)
            drain_inst.ins.sync_info = bass_rust.SyncInfo(
                on_wait=[waits[0]], on_update=list(si.on_update)
            )
            for w in waits[1:]:
                nop = self.nc.sync.nop(nofuse=True)
                nop.ins.sync_info = bass_rust.SyncInfo(on_wait=[w], on_update=[])
        self.nc.all_engine_barrier()
        assert self.sems is not None
        popped = self.nc._tile_sem_poison_stack.pop()
        assert popped is self._sem_poison
        self.nc.clear_and_free_semaphores(list(self.sems.allocated().values()))
        self.nc.all_engine_barrier()

    TileContext._drain_and_barrier = _drain_and_barrier
    TileContext._drain_patched = True


def build_gat_nc(N=8192, R=1024, FIN=256, FOUT=128):
    """Build the per-core Bass program (transposed layout). All cores run the
    same program on different data slices."""
    import os

    # bisection knobs (default = fastest path)
    swdge_split = int(os.environ.get("GAT_SWDGE_SPLIT", "2"))  # chunks per accum DMA (4=whole group fails >4KB/partition)
    no_pool_tt = bool(int(os.environ.get("GAT_NO_POOL_TT", "1")))
    no_dma_mask = bool(int(os.environ.get("GAT_NO_DMA_MASK", "0")))
    _patch_tail_drain()

    P = 128
    FK = FIN // P          # fin chunks (contraction for Wh)
    NCH = N // P           # 128-row j-chunks over all N source nodes
    RB = R // P            # 128-wide i-subblocks per core
    SEG = 512 if R % 512 == 0 else R
    NSEG = R // SEG
    EB = 2 if NCH % 2 == 0 else 1   # chunks per batched Exp / mask-DMA group
    WB = 2 if NCH % 2 == 0 else 1   # Wh chunks per PSUM tile

    nc = bass.Bass()
    hT_t = nc.dram_tensor("hT", [FIN, N], BF16, kind="ExternalInput")
    hTo_t = nc.dram_tensor("hT_own", [FIN, R], BF16, kind="ExternalInput")
    # mask, fp8 {0,-64}, pre-arranged so group G lives at rows [G*128,(G+1)*128)
    # with the EB chunks of the group concatenated along the free dim.
    madj_t = nc.dram_tensor("madj8", [(NCH // EB) * P, EB * R], FP8, kind="ExternalInput")
    w_t = nc.dram_tensor("W", [FOUT, FIN], F32, kind="ExternalInput")
    wT_t = nc.dram_tensor("WT", [FIN, FOUT], BF16, kind="ExternalInput")
    a_t = nc.dram_tensor("a", [2 * FOUT, 1], F32, kind="ExternalInput")
    out_t = nc.dram_tensor("out_blk", [R, FOUT], F32, kind="ExternalOutput")

    with tile.TileContext(nc) as tc:
        with tc.tile_pool(name="persist", bufs=1) as persist:
            ident = persist.tile([P, P], F32)
            make_identity(nc, ident)
            ones_col = persist.tile([P, 1], BF16)
            nc.vector.memset(ones_col, 1.0)
            ones_row = persist.tile([1, P], BF16)
            nc.vector.memset(ones_row, 1.0)
            hT_sb = persist.tile([P, FK, N], BF16)       # h.T, fin on partitions
            hTo_sb = persist.tile([P, FK, R], BF16)      # own rows of h.T
            whs_sb = persist.tile([P, NCH, FOUT], BF16)  # Wh, j on partitions
            sdst_col = persist.tile([P, NCH, 1], F32)    # s_dst, partition-major
            ssrc_bcast = persist.tile([P, R], BF16)      # s_src bcast to all partitions
            ssrc02_bcast = persist.tile([P, R], BF16)    # 0.2 * s_src bcast
            sdst02_col = persist.tile([P, NCH, 1], F32)  # 0.2 * s_dst
            rhs_aug = persist.tile([P, FK, FOUT + 1], BF16)  # [W.T | w_dst] per fin chunk
            wsrc_sb = persist.tile([P, FK], BF16)        # w_src per fin chunk

            # startup DMAs: spread dispatch across engine sequencers (each
            # HWDGE dispatch costs ~600ns of sequencer time; serializing 20+
            # of them on SP alone wasted 14us of startup).
            for k in range(FK):
                nc.scalar.dma_start(out=hTo_sb[:, k, :], in_=hTo_t[k * P : (k + 1) * P, :])
                nc.scalar.dma_start(
                    out=rhs_aug[:, k, 0:FOUT], in_=wT_t[k * P : (k + 1) * P, :]
                )
            HPC = N // 2 if N % 2 == 0 else N
            for c0 in range(0, N, HPC):
                for k in range(FK):
                    nc.sync.dma_start(
                        out=hT_sb[:, k, c0 : c0 + HPC],
                        in_=hT_t[k * P : (k + 1) * P, c0 : c0 + HPC],
                    )

            # ---------------- prologue: w_src/w_dst, s_src ----------------
            with (
                tc.tile_pool(name="pro", bufs=1) as pro,
                tc.tile_pool(name="pro_ps", bufs=1, space="PSUM") as pro_ps,
            ):
                w_sb = pro.tile([P, FIN], F32)
                nc.scalar.dma_start(out=w_sb, in_=w_t[:, :])
                acol = pro.tile([P, 2], F32)
                nc.scalar.dma_start(out=acol[:, 0:1], in_=a_t[0:FOUT, :])        # a_src
                nc.scalar.dma_start(out=acol[:, 1:2], in_=a_t[FOUT : 2 * FOUT, :])  # a_dst

                for k in range(FK):
                    wchunk = w_sb[:, k * P : (k + 1) * P]
                    pw = pro_ps.tile([P, 2], F32, tag="wv")
                    nc.tensor.matmul(pw[:, 0:1], wchunk, acol[:, 1:2], start=True, stop=True)
                    nc.tensor.matmul(pw[:, 1:2], wchunk, acol[:, 0:1], start=True, stop=True)
                    nc.vector.tensor_copy(out=rhs_aug[:, k, FOUT : FOUT + 1], in_=pw[:, 0:1])
                    nc.vector.tensor_copy(out=wsrc_sb[:, k : k + 1], in_=pw[:, 1:2])

                # s_src for own rows (bf16 operands, fp32 PSUM accumulate)
                sp = pro_ps.tile([P, RB], F32, tag="sp")
                for b in range(RB):
                    for k in range(FK):
                        nc.tensor.matmul(
                            sp[:, b : b + 1],
                            hTo_sb[:, k, b * P : (b + 1) * P],
                            wsrc_sb[:, k : k + 1],
                            start=(k == 0),
                            stop=(k == FK - 1),
                        )
                ssrc_col = pro.tile([P, RB], F32)
                nc.vector.tensor_copy(out=ssrc_col, in_=sp)

                # s_src broadcast across partitions: per-partition columns ->
                # one row (PE transposes), then outer-product with ones (K=1
                # matmul) to replicate down the partition dim.
                srow_ps = pro_ps.tile([1, R], F32, tag="srow")
                for b in range(RB):
                    nc.tensor.transpose(
                        srow_ps[:, b * P : (b + 1) * P], ssrc_col[:, b : b + 1], ident
                    )
                srow_sb = pro.tile([1, R], BF16)
                nc.vector.tensor_copy(out=srow_sb, in_=srow_ps)
                sbc_ps = pro_ps.tile([P, R], F32, tag="sbc")
                BSEG = 512 if R % 512 == 0 else R
                for s in range(R // BSEG):
                    nc.tensor.matmul(
                        sbc_ps[:, s * BSEG : (s + 1) * BSEG],
                        ones_row,
                        srow_sb[:, s * BSEG : (s + 1) * BSEG],
                        start=True,
                        stop=True,
                    )
                nc.vector.tensor_copy(out=ssrc_bcast, in_=sbc_ps)
                nc.vector.tensor_scalar(
                    out=ssrc02_bcast, in0=ssrc_bcast, scalar1=0.2, scalar2=None,
                    op0=ALU.mult,
                )

            # ------------- main: Wh chunks interleaved with attention -------------
            with (
                tc.tile_pool(name="whp", bufs=2, space="PSUM") as whp,
                tc.tile_pool(name="e1p", bufs=6) as e1p,
                tc.tile_pool(name="e2p", bufs=6) as e2p,
                tc.tile_pool(name="xbp", bufs=6) as xbp,
                tc.tile_pool(name="pqp", bufs=6) as pqp,
                tc.tile_pool(name="sm", bufs=2) as sm,
                tc.tile_pool(name="osb", bufs=2) as osb,
                tc.tile_pool(name="out_ps", bufs=1, space="PSUM") as out_ps,
                tc.tile_pool(name="tr_ps", bufs=1, space="PSUM") as tr_ps,
            ):
                psum_outT = [
                    out_ps.tile([P, SEG], F32, tag=f"poT{s}", name=f"poT{s}")
                    for s in range(NSEG)
                ]
                psum_sums = [
                    out_ps.tile([1, SEG], F32, tag=f"psm{s}", name=f"psm{s}")
                    for s in range(NSEG)
                ]
                xb = None
                pending = None
                flush_ref = []
                # ---- Wh phase: all chunks up front (fills the startup DMA
                # window; keeps PSUM-copy traffic out of the attention loop
                # so the DVE/ACT FIFOs never stall on cross-phase deps) ----
                for c2 in range(NCH // WB):
                    wh_ps = whp.tile([P, WB, FOUT + 1], F32, tag="wh", name="wh_ps")
                    for i in range(WB):
                        c = c2 * WB + i
                        for k in range(FK):
                            nc.tensor.matmul(
                                wh_ps[:, i, :],
                                hT_sb[:, k, c * P : (c + 1) * P],
                                rhs_aug[:, k, :],
                                start=(k == 0),
                                stop=(k == FK - 1),
                            )
                    if c2 % 2 == 0:
                        nc.vector.tensor_copy(
                            out=whs_sb[:, c2 * WB : (c2 + 1) * WB, :],
                            in_=wh_ps[:, :, 0:FOUT],
                        )
                    else:
                        nc.scalar.activation(
                            out=whs_sb[:, c2 * WB : (c2 + 1) * WB, :],
                            in_=wh_ps[:, :, 0:FOUT],
                            func=AF.Copy,
                            bias=0.0,
                        )
                    nc.scalar.activation(
                        out=sdst_col[:, c2 * WB : (c2 + 1) * WB, :],
                        in_=wh_ps[:, :, FOUT : FOUT + 1],
                        func=AF.Copy,
                        bias=0.0,
                    )
                    nc.scalar.activation(
                        out=sdst02_col[:, c2 * WB : (c2 + 1) * WB, :],
                        in_=wh_ps[:, :, FOUT : FOUT + 1],
                        func=AF.Copy,
                        bias=0.0,
                        scale=0.2,
                    )

                # ---- attention loop ----
                for c2 in range(NCH // WB):
                    def flush_group(jc_last, xbt):
                        """Mask + exp + matmuls for the EB-chunk group ending
                        at jc_last. Emitted one group late (software pipeline)
                        so the Pool/ACT FIFOs never head-of-line block on the
                        mask DMA's dependencies."""
                        grp = jc_last // EB
                        if no_dma_mask:
                            mtl = e1p.tile([P, EB, R], FP8, tag="mt", name="mtl")
                            nc.sync.dma_start(
                                out=mtl, in_=madj_t[grp * P : (grp + 1) * P, :]
                            )
                            for gg in range(EB):
                                nc.vector.tensor_tensor(
                                    out=xbt[:, gg, :], in0=xbt[:, gg, :],
                                    in1=mtl[:, gg, :], op=ALU.add,
                                )
                        else:
                            sw = swdge_split if swdge_split else EB
                            for gg in range(0, EB, sw):
                                nc.gpsimd.dma_start(
                                    out=xbt[:, gg : gg + sw, :],
                                    in_=madj_t[
                                        grp * P : (grp + 1) * P,
                                        gg * R : (gg + sw) * R,
                                    ],
                                    accum_op=ALU.add,
                                )
                        pq = pqp.tile([P, EB, R], BF16, tag="pq", name="pq")
                        nc.scalar.activation(out=pq, in_=xbt, func=AF.Exp)
                        jc0 = jc_last - (EB - 1)
                        for gg in range(EB):
                            jcc = jc0 + gg
                            for s in range(NSEG):
                                nc.tensor.matmul(
                                    psum_outT[s],
                                    whs_sb[:, jcc, :],
                                    pq[:, gg, s * SEG : (s + 1) * SEG],
                                    start=(jcc == 0),
                                    stop=(jcc == NCH - 1),
                                )
                        for gg in range(EB):
                            jcc = jc0 + gg
                            for s in range(NSEG):
                                nc.tensor.matmul(
                                    psum_sums[s],
                                    ones_col,
                                    pq[:, gg, s * SEG : (s + 1) * SEG],
                                    start=(jcc == 0),
                                    stop=(jcc == NCH - 1),
                                )

                    flush_ref[:] = [flush_group]

                    for i in range(WB):
                        jc = c2 * WB + i
                        g = jc % EB
                        if g == 0:
                            xb = xbp.tile([P, EB, R], BF16, tag="xb", name="xb")
                        if jc % 16 in (3, 7, 11, 15):
                            # balance valve: full leakyrelu on ACT (bias and
                            # alpha fused into one ACTIVATE)
                            nc.scalar.activation(
                                out=xb[:, g, :],
                                in_=ssrc_bcast,
                                func=AF.Prelu,
                                bias=sdst_col[:, jc, :],
                                scale=1.0,
                                alpha=0.2,
                            )
                        else:
                            # e1 = s_src + s_dst[j] (Pool TT-add with a
                            # free-broadcast [P,1] operand for 4/16 chunks,
                            # DVE TS otherwise); e2 = 0.2*e1 (single-op TS);
                            # leakyrelu = max(e1, e2) on DVE.
                            e1 = e1p.tile([P, R], BF16, tag="e1", name="e1")
                            if not no_pool_tt and jc % 16 in (0, 2, 5, 9, 13):
                                nc.gpsimd.tensor_tensor(
                                    out=e1,
                                    in0=ssrc_bcast,
                                    in1=bass.broadcast_tensor_aps(
                                        ssrc_bcast[:, :], sdst_col[:, jc, :]
                                    )[1],
                                    op=ALU.add,
                                )
                            else:
                                nc.vector.tensor_scalar(
                                    out=e1,
                                    in0=ssrc_bcast,
                                    scalar1=sdst_col[:, jc, :],
                                    scalar2=None,
                                    op0=ALU.add,
                                )
                            e2 = e2p.tile([P, R], BF16, tag="e2", name="e2")
                            nc.vector.tensor_scalar(
                                out=e2,
                                in0=ssrc02_bcast,
                                scalar1=sdst02_col[:, jc, :],
                                scalar2=None,
                                op0=ALU.add,
                            )
                            nc.vector.tensor_tensor(
                                out=xb[:, g, :], in0=e1, in1=e2, op=ALU.max
                            )
                        if g != EB - 1:
                            continue
                        if pending is not None:
                            flush_group(*pending)
                        pending = (jc, xb)

                if pending is not None:
                    flush_ref[0](*pending)

                # tail: denominators back to per-partition layout, transpose
                # out.T blocks, scale, store.
                sums_sb = sm.tile([1, R], F32, tag="ssb", name="sums_sb")
                for s in range(NSEG):
                    nc.vector.tensor_copy(
                        out=sums_sb[:, s * SEG : (s + 1) * SEG], in_=psum_sums[s]
                    )
                # [1, R] row -> [P, RB] per-partition columns via tiny PE
                # transposes ([1,128].T @ [[1]] = [128,1]).
                rsums_ps = tr_ps.tile([P, RB], F32, tag="rs", name="rsums_ps")
                for b in range(RB):
                    nc.tensor.transpose(
                        rsums_ps[:, b : b + 1],
                        sums_sb[0:1, b * P : (b + 1) * P],
                        ident[0:1, 0:1],
                    )
                recip_col = sm.tile([P, RB], F32, tag="rcc", name="recip_col")
                nc.vector.reciprocal(recip_col, rsums_ps)
                outT_sb = sm.tile([P, R], F32, tag="oT", name="outT_sb")
                for s in range(NSEG):
                    nc.vector.tensor_copy(
                        out=outT_sb[:, s * SEG : (s + 1) * SEG], in_=psum_outT[s]
                    )
                for b in range(RB):
                    tr = tr_ps.tile([P, P], F32, tag="tr", name="tr")
                    nc.tensor.transpose(
                        tr, outT_sb[:, b * P : (b + 1) * P], ident
                    )
                    out_sb = osb.tile([P, FOUT], F32, tag="ob", name="out_sb")
                    nc.scalar.activation(
                        out=out_sb,
                        in_=tr,
                        func=AF.Copy,
                        bias=0.0,
                        scale=recip_col[:, b : b + 1],
                    )
                    nc.sync.dma_start(out=out_t[b * P : (b + 1) * P, :], in_=out_sb)

    return nc


@functools.lru_cache(maxsize=2)
def _compiled(N, R, FIN, FOUT):
    return build_gat_nc(N=N, R=R, FIN=FIN, FOUT=FOUT)


def run_gat(h, adj, W, a, trace=False, tmpdir=None):
    BF = ml_dtypes.bfloat16
    E4 = ml_dtypes.float8_e4m3
    h = np.asarray(h, dtype=np.float32)
    adj = np.asarray(adj, dtype=np.int32)
    N, FIN = h.shape
    FOUT = np.asarray(W).shape[0]
    R = N // N_CORES
    P = 128
    NCH = N // P
    EB = 2 if NCH % 2 == 0 else 1
    nc = _compiled(N, R, FIN, FOUT)

    hT_bf = np.ascontiguousarray(h.T.astype(BF))
    W32 = np.ascontiguousarray(np.asarray(W, dtype=np.float32))
    WT_bf = np.ascontiguousarray(W32.T.astype(BF))
    a32 = np.ascontiguousarray(np.asarray(a, dtype=np.float32).reshape(2 * FOUT, 1))
    # additive mask: adj==1 -> 0.0, adj==0 -> MASK_NEG, fp8_e4m3
    lut = np.array([MASK_NEG, 0.0], dtype=E4)

    in_maps = []
    for c in range(N_CORES):
        sl = slice(c * R, (c + 1) * R)
        madjT = lut[adj[sl].T]                    # [N, R] fp8 {0,-64}
        # group-major layout: [NCH//EB, EB, P, R] -> [NCH//EB, P, EB, R]
        m8 = (
            madjT.reshape(NCH // EB, EB, P, R)
            .transpose(0, 2, 1, 3)
            .reshape((NCH // EB) * P, EB * R)
        )
        in_maps.append(
            {
                "hT": hT_bf,
                "hT_own": np.ascontiguousarray(h[sl].T.astype(BF)),
                "madj8": np.ascontiguousarray(m8),
                "W": W32,
                "WT": WT_bf,
                "a": a32,
            }
        )
    res = run_bass_kernel_spmd(
        nc, in_maps, core_ids=list(range(N_CORES)), trace=trace, tmpdir=tmpdir
    )
    out = np.concatenate([r["out_blk"] for r in res.results], axis=0)
    return out, res


def kernel(h, adj, W, a):
    out, _ = run_gat(np.asarray(h), np.asarray(adj), np.asarray(W), np.asarray(a))
    return out.astype(np.float32)
